# revision 15
# baseline (speedup 1.0000x reference)
"""Distributed CLIP-style symmetric InfoNCE loss on 8 trn2 NeuronCores.

reference semantics:
    xn = x / ||x||_row ; yn = y / ||y||_row
    logits = (xn @ yn.T) / 0.01                     # [N, N]
    loss = 0.5*(CE(logits, diag) + CE(logits.T, diag))
         = mean_i( lse(logits[i,:]) - logits[i,i] )/2
         + mean_j( lse(logits[:,j]) - logits[j,j] )/2

Sharding: row-parallel.  Core i owns x rows [i*R, (i+1)*R) and a copy of y
ROTATED by -i*R rows, so the global diagonal element for local row r sits at
local column r on every core -> one SPMD program, no collectives.

Each core computes (no max-subtraction needed: |logits| <= ~20 so exp() is
safe in f32):
    rowsum[r]  = sum_c exp(l[r,c])      (full row,   c over all N columns)
    colpart[c] = sum_r exp(l[r,c])      (partial col, r over the R local rows)
    diag[r]    = l[r,r]
Host: colsum = sum_i roll(colpart_i, i*R); loss from log(rowsum), log(colsum),
diag in float64.

Matmul path: fp16 operands (PE: 1 cycle/row like bf16, but 2^-12 rel error).
Row norms are computed in f32 from the raw tiles; the normalization scales are
folded into the PE transposes by replacing the identity moving-operand with a
diagonal scale matrix, so normalize+transpose is a single PE op.
"""

import numpy as np

import concourse.bacc as bacc
import concourse.bass as bass
import concourse.mybir as mybir
import concourse.tile as tile
from concourse.bass_utils import run_bass_kernel_spmd

F32 = mybir.dt.float32
FP16 = mybir.dt.float16
I32 = mybir.dt.int32
AF = mybir.ActivationFunctionType
ALU = mybir.AluOpType

N_CORES = 8
N_FULL, D_FULL = 8192, 1024
P = 128
NCHUNK = 512  # y rows per matmul moving chunk (one PSUM bank of f32)
TEMP_INV = 100.0  # 1/TEMPERATURE


def build_nc(R=N_FULL // N_CORES, C=N_FULL, D=D_FULL, mm_dt=FP16):
    """Build the single-core Bass program (SPMD across cores via inputs)."""
    NK = D // P      # contraction tiles
    RM = R // P      # x-row tiles (stationary/M blocks)
    NCH = C // NCHUNK  # y-row chunks (moving/N blocks)
    assert NCH % 2 == 0, "chunk pairing assumes an even chunk count"
    JB = NCHUNK // P  # 128-row blocks per y chunk

    nc = bacc.Bacc(None, target_bir_lowering=False)

    x_d = nc.dram_tensor("x_slab", [R, D], F32, kind="ExternalInput")
    y_d = nc.dram_tensor("y_full", [C, D], F32, kind="ExternalInput")
    rowsum_d = nc.dram_tensor("rowsum", [P, RM], F32, kind="ExternalOutput")
    diag_d = nc.dram_tensor("diag", [P, RM], F32, kind="ExternalOutput")
    colsum_d = nc.dram_tensor("colsum", [1, C], F32, kind="ExternalOutput")

    with tile.TileContext(nc) as tc:
        with (
            tc.tile_pool(name="const", bufs=1) as p_const,
            tc.tile_pool(name="persist", bufs=1) as p_persist,
            tc.tile_pool(name="stats", bufs=2) as p_stats,
            tc.tile_pool(name="sq", bufs=1) as p_sq,
            tc.tile_pool(name="ytiles", bufs=8) as p_y,
            tc.tile_pool(name="ytiles16", bufs=8) as p_y16,
            tc.tile_pool(name="ynt", bufs=4) as p_ynt,
            tc.tile_pool(name="etile", bufs=4) as p_e,
            tc.tile_pool(name="psmm", bufs=4, space="PSUM") as p_mm,
            tc.tile_pool(name="psytr", bufs=2, space="PSUM") as p_ytr,
        ):
            # ---- constants: identity mask I[p, f] = (p == f) in f32 ----
            iot = p_const.tile([P, P], I32)
            ident = p_const.tile([P, P], F32)
            ident16 = p_const.tile([P, P], mm_dt)
            nc.gpsimd.iota(iot[:, :], pattern=[[1, P]], base=0, channel_multiplier=-1)
            nc.vector.tensor_scalar(
                ident[:, :], iot[:, :], 0, None, op0=ALU.is_equal
            )
            nc.scalar.copy(ident16[:, :], ident[:, :])

            # ---- persistent accumulators ----
            xnt = p_persist.tile([P, NK * R], mm_dt)       # x^T, normalized*100
            colacc = p_persist.tile([P, C], F32)           # per-partition col sums
            rowacc = p_persist.tile([P, RM * NCH], F32)    # per (m, chunk) row sums
            rowsum_t = p_persist.tile([P, RM], F32)
            diag_t = p_persist.tile([P, RM], F32)
            colfin = p_persist.tile([1, C], F32)
            ones_t = p_const.tile([P, 1], F32)
            nc.vector.memset(colacc[:, :], 0.0)
            nc.vector.memset(ones_t[:, :], 1.0)

            # ---- x phase: load, row norms, fold 100/||x|| into transpose ----
            with (
                tc.tile_pool(name="xtiles", bufs=3) as p_x,
                tc.tile_pool(name="xtiles16", bufs=3) as p_x16,
                tc.tile_pool(name="psxtr", bufs=2, space="PSUM") as p_xtr,
            ):
                ssx = p_stats.tile([P, RM], F32, tag="ss")
                rxx = p_stats.tile([P, RM], F32, tag="rx")
                sxs = p_stats.tile([P, RM], F32, tag="s")
                sq = p_sq.tile([P, D], F32, tag="sq")
                for m in range(RM):
                    xr = p_x.tile([P, D], F32, tag="xr")
                    nc.sync.dma_start(out=xr[:, :], in_=x_d[m * P:(m + 1) * P, :])
                    nc.vector.scalar_tensor_tensor(
                        out=sq[:, :], in0=xr[:, :], scalar=1.0, in1=xr[:, :],
                        op0=ALU.mult, op1=ALU.mult,
                        accum_out=ssx[:, m:m + 1],
                    )
                    nc.vector.reciprocal(rxx[:, m:m + 1], ssx[:, m:m + 1])
                    # sqrt(10000/ss) = 100/||x||
                    nc.scalar.activation(
                        sxs[:, m:m + 1], rxx[:, m:m + 1], AF.Sqrt,
                        scale=float(TEMP_INV * TEMP_INV),
                    )
                    # normalize+scale+cast in one ACT pass
                    xr16 = p_x16.tile([P, D], mm_dt, tag="xr16")
                    nc.scalar.activation(
                        xr16[:, :], xr[:, :], AF.Copy, scale=sxs[:, m:m + 1]
                    )
                    for k in range(NK):
                        ps = p_xtr.tile([P, P], mm_dt, tag="xtr")
                        nc.tensor.transpose(
                            ps[:, :], xr16[:, k * P:(k + 1) * P], ident16[:, :]
                        )
                        nc.scalar.copy(
                            xnt[:, k * R + m * P: k * R + (m + 1) * P], ps[:, :]
                        )

            # ---- main loop over chunk pairs ----
            for pair in range(NCH // 2):
                ynt_pair = []
                for c2 in range(2):
                    n = 2 * pair + c2
                    ssy = p_stats.tile([P, JB], F32, tag="ssy")
                    rxy = p_stats.tile([P, JB], F32, tag="rxy")
                    sy = p_stats.tile([P, JB], F32, tag="sy")
                    ynt = p_ynt.tile([P, NK * NCHUNK], mm_dt, tag="ynt")
                    ynt_pair.append(ynt)
                    yrs = []
                    for j in range(JB):
                        r0 = n * NCHUNK + j * P
                        yr = p_y.tile([P, D], F32, tag="yr")
                        nc.sync.dma_start(out=yr[:, :], in_=y_d[r0:r0 + P, :])
                        sq = p_sq.tile([P, D], F32, tag="sq")
                        nc.vector.scalar_tensor_tensor(
                            out=sq[:, :], in0=yr[:, :], scalar=1.0, in1=yr[:, :],
                            op0=ALU.mult, op1=ALU.mult,
                            accum_out=ssy[:, j:j + 1],
                        )
                        yrs.append(yr)
                    nc.vector.reciprocal(rxy[:, :], ssy[:, :])
                    nc.scalar.activation(sy[:, :], rxy[:, :], AF.Sqrt)
                    yr16s = []
                    for j in range(JB):
                        yr16 = p_y16.tile([P, D], mm_dt, tag="yr16")
                        nc.scalar.activation(
                            yr16[:, :], yrs[j][:, :], AF.Copy, scale=sy[:, j:j + 1]
                        )
                        yr16s.append(yr16)
                    # transpose k-major: psum tile [P, NCHUNK] per k
                    for k in range(NK):
                        ps = p_ytr.tile([P, NCHUNK], mm_dt, tag="ytr")
                        for j in range(JB):
                            nc.tensor.transpose(
                                ps[:, j * P:(j + 1) * P],
                                yr16s[j][:, k * P:(k + 1) * P],
                                ident16[:, :],
                            )
                        nc.scalar.copy(
                            ynt[:, k * NCHUNK:(k + 1) * NCHUNK], ps[:, :]
                        )

                # matmuls: stationary (m,k) reused across the chunk pair
                for m in range(RM):
                    pmm = [
                        p_mm.tile([P, NCHUNK], F32, tag="mm", name=f"pmm{c2}")
                        for c2 in range(2)
                    ]
                    for k in range(NK):
                        for c2 in range(2):
                            nc.tensor.matmul(
                                pmm[c2][:, :],
                                xnt[:, k * R + m * P: k * R + (m + 1) * P],
                                ynt_pair[c2][:, k * NCHUNK:(k + 1) * NCHUNK],
                                start=(k == 0),
                                stop=(k == NK - 1),
                            )
                    for c2 in range(2):
                        n = 2 * pair + c2
                        # diagonal block of this core sits at local col == local row
                        if (m * P) // NCHUNK == n:
                            off = (m * P) % NCHUNK
                            dsc = p_sq.tile([P, P], F32, tag="dscratch")
                            nc.vector.scalar_tensor_tensor(
                                out=dsc[:, :],
                                in0=pmm[c2][:, off:off + P],
                                scalar=1.0,
                                in1=ident[:, :],
                                op0=ALU.mult, op1=ALU.mult,
                                accum_out=diag_t[:, m:m + 1],
                            )
                        et = p_e.tile([P, NCHUNK], F32, tag="E")
                        nc.scalar.activation(
                            et[:, :], pmm[c2][:, :], AF.Exp,
                            accum_out=rowacc[:, m * NCH + n: m * NCH + n + 1],
                        )
                        nc.vector.tensor_add(
                            colacc[:, n * NCHUNK:(n + 1) * NCHUNK],
                            colacc[:, n * NCHUNK:(n + 1) * NCHUNK],
                            et[:, :],
                        )

                # chunks of this pair are fully accumulated: reduce the 128
                # colacc partitions with a ones-matmul (f32, overlaps next pair)
                for c2 in range(2):
                    n = 2 * pair + c2
                    pcs = p_ytr.tile([1, NCHUNK], F32, tag="ytr", name="pcs")
                    nc.tensor.matmul(
                        pcs[:, :],
                        ones_t[:, :],
                        colacc[:, n * NCHUNK:(n + 1) * NCHUNK],
                        start=True,
                        stop=True,
                    )
                    nc.scalar.copy(
                        colfin[0:1, n * NCHUNK:(n + 1) * NCHUNK], pcs[:, :]
                    )

            # ---- finalize ----
            for m in range(RM):
                nc.vector.reduce_sum(
                    rowsum_t[:, m:m + 1],
                    rowacc[:, m * NCH:(m + 1) * NCH],
                    axis=mybir.AxisListType.X,
                )
            nc.sync.dma_start(out=rowsum_d[:, :], in_=rowsum_t[:, :])
            nc.sync.dma_start(out=diag_d[:, :], in_=diag_t[:, :])
            nc.sync.dma_start(out=colsum_d[:, :], in_=colfin[0:1, :])

    # bacc passes legalize the schedule for walrus (notably: split multi-sem
    # waits -- TRN2 instructions carry at most one wait condition)
    nc.compile()
    return nc


_NC_CACHE = {}


def get_nc(R=N_FULL // N_CORES, C=N_FULL, D=D_FULL, mm_dt=FP16):
    key = (R, C, D, str(mm_dt))
    if key not in _NC_CACHE:
        _NC_CACHE[key] = build_nc(R, C, D, mm_dt)
    return _NC_CACHE[key]


def make_in_maps(x, y, n_cores=N_CORES):
    R = x.shape[0] // n_cores
    maps = []
    for i in range(n_cores):
        maps.append({
            "x_slab": np.ascontiguousarray(x[i * R:(i + 1) * R]),
            "y_full": np.ascontiguousarray(np.roll(y, -i * R, axis=0)),
        })
    return maps


def finish_host(core_outs, n_cores=N_CORES):
    """core_outs: list of dicts with rowsum [P,RM], diag [P,RM], colsum [1,C]."""
    rows, diags = [], []
    colsum = None
    for i, out in enumerate(core_outs):
        R = out["rowsum"].shape[0] * out["rowsum"].shape[1]
        rows.append(out["rowsum"].astype(np.float64).T.reshape(-1))
        diags.append(out["diag"].astype(np.float64).T.reshape(-1))
        part = np.roll(out["colsum"].reshape(-1).astype(np.float64), i * R)
        colsum = part if colsum is None else colsum + part
    rowsum = np.concatenate(rows)
    diag = np.concatenate(diags)
    loss = 0.5 * ((np.log(rowsum) - diag).mean() + (np.log(colsum) - diag).mean())
    return np.float32(loss)


def kernel(x, y):
    x = np.ascontiguousarray(np.asarray(x, dtype=np.float32))
    y = np.ascontiguousarray(np.asarray(y, dtype=np.float32))
    nc = get_nc()
    res = run_bass_kernel_spmd(nc, make_in_maps(x, y), core_ids=list(range(N_CORES)))
    return finish_host(res.results)
